# revision 2
# baseline (speedup 1.0000x reference)
"""Causal single-head attention (B=4, S=2048, D=1024) on 8 NeuronCores.

Sharding: each core owns one (batch, parity) pair — core c handles the
q rows {2i + (c%2)} of batch c//2 (1024 rows). Interleaving q rows by
parity gives every core an identical causal block structure, so one
SPMD program serves all 8 cores; only the input data (and the staircase
mask) differs per core.

Key layout trick: each core's copy of x[b] is row-permuted to
[own-parity rows asc, other-parity rows asc]. Attention is invariant to
key order, so K/V computed from the permuted x just need a matching
mask. The core's q rows are then a contiguous prefix (rows 0:1024) of
its permuted x, and the causal extent of q-block j (512 rows) becomes a
uniform set of key tiles: [0, 4(j+1)) and [8, 8+4(j+1)) in 128-key
tiles, with exactly 8 diagonal-crossing tiles per block.

Device program per core (all matmuls bf16 with f32 PSUM accumulation):
  kT[e,s]  = (x @ Wk).T      scoresT[sk,sq] = kT.T-tiles @ qT-tiles
  v[s,e]   = x @ Wv          w = exp(scoresT/32) * mask   (no max-sub:
  qT[e,i]  = (xq @ Wq).T      |scores/32| <~ 6, exp is safe in f32)
  out[sq,e] = (w.T-tiles @ v) / (w.T @ ones)   (rowsum via N=1 matmul)
"""

import sys
import types

import numpy as np
import ml_dtypes

import concourse.tile as tile
from concourse import bacc, mybir
from concourse.bass_utils import run_bass_kernel_spmd


def _ensure_ntff_hook():
    """bass_utils imports antenv.axon_hooks when tracing; some containers
    lack that module. Register a process-local equivalent so trace=True
    works (or degrades to untraced instead of crashing)."""
    try:
        import antenv.axon_hooks  # noqa: F401
        return
    except ImportError:
        pass
    hook = None
    try:
        from trn_agent_boot.trn_boot import _ntff_profile_via_ctypes
        hook = _ntff_profile_via_ctypes("/opt/axon/libaxon_pjrt.so")
    except Exception:
        hook = None
    mod = types.ModuleType("antenv.axon_hooks")
    mod.get_axon_ntff_profile_hook = lambda: hook
    mod.set_axon_ntff_profile_hook = lambda h: None
    sys.modules["antenv.axon_hooks"] = mod


_ensure_ntff_hook()

BF16 = mybir.dt.bfloat16
F32 = mybir.dt.float32
AF = mybir.ActivationFunctionType

B, S, D = 4, 2048, 1024
P = 128
NCORES = 8
SQ = 1024            # q rows per core
ND = D // P          # 8 contraction tiles over d
NE = D // P          # 8 tiles over e (d_out)
NSK = S // P         # 16 key tiles
QB = 512             # q-block width (matmul free dim)
NQB = SQ // QB       # 2 q blocks
SCALE = 1.0 / np.sqrt(np.float32(D))

TRACE = False
LAST_RESULT = None

_cache = {}


def _sk_list(j):
    # key tiles needed by q-block j: prefix of each parity half
    return list(range(0, 4 * (j + 1))) + list(range(8, 8 + 4 * (j + 1)))


def _cross_list(j):
    # diagonal-crossing key tiles of q-block j (order matches maskd[j])
    return list(range(4 * j, 4 * (j + 1))) + list(range(8 + 4 * j, 8 + 4 * (j + 1)))


def _build():
    nc = bacc.Bacc("TRN2", target_bir_lowering=False, debug=False,
                   num_devices=NCORES)
    xfT = nc.dram_tensor("xfT", [D, S], BF16, kind="ExternalInput")
    wq = nc.dram_tensor("wq", [D, D], BF16, kind="ExternalInput")
    wk = nc.dram_tensor("wk", [D, D], BF16, kind="ExternalInput")
    wv = nc.dram_tensor("wv", [D, D], BF16, kind="ExternalInput")
    maskd = nc.dram_tensor("maskd", [NQB, 8, P, QB], BF16, kind="ExternalInput")
    ones = nc.dram_tensor("ones", [P, 8], BF16, kind="ExternalInput")
    out = nc.dram_tensor("out", [SQ, D], F32, kind="ExternalOutput")

    with tile.TileContext(nc) as tc:
        with (
            tc.tile_pool(name="xf", bufs=ND) as xf_pool,
            tc.tile_pool(name="w", bufs=2 * ND) as w_pool,
            tc.tile_pool(name="kT", bufs=NE) as kT_pool,
            tc.tile_pool(name="v", bufs=NSK) as v_pool,
            tc.tile_pool(name="qT", bufs=NE) as qT_pool,
            tc.tile_pool(name="mk", bufs=2 * 8) as m_pool,
            tc.tile_pool(name="we", bufs=16) as we_pool,
            tc.tile_pool(name="on", bufs=1) as on_pool,
            tc.tile_pool(name="sm", bufs=4) as sm_pool,
            tc.tile_pool(name="o", bufs=2) as o_pool,
            tc.tile_pool(name="ps", bufs=2, space="PSUM") as ps_pool,
            tc.tile_pool(name="av", bufs=2, space="PSUM") as av_pool,
            tc.tile_pool(name="rs", bufs=2, space="PSUM") as rs_pool,
        ):
            # ---- input DMAs (xf + wk first: stage A's critical path) ----
            xf = []
            for d in range(ND):
                t = xf_pool.tile([P, S], BF16, tag="xf")
                nc.sync.dma_start(t[:], xfT[d * P:(d + 1) * P, :])
                xf.append(t)
            wk_t = []
            for d in range(ND):
                t = w_pool.tile([P, D], BF16, tag="w")
                nc.sync.dma_start(t[:], wk[d * P:(d + 1) * P, :])
                wk_t.append(t)
            wv_t = []
            for d in range(ND):
                t = w_pool.tile([P, D], BF16, tag="w")
                nc.sync.dma_start(t[:], wv[d * P:(d + 1) * P, :])
                wv_t.append(t)
            wq_t = []
            for d in range(ND):
                t = w_pool.tile([P, D], BF16, tag="w")
                nc.sync.dma_start(t[:], wq[d * P:(d + 1) * P, :])
                wq_t.append(t)
            m_t = [[None] * 8 for _ in range(NQB)]
            for j in range(NQB):
                for tt in range(8):
                    t = m_pool.tile([P, QB], BF16, tag="mk")
                    nc.sync.dma_start(t[:], maskd[j, tt, :, :])
                    m_t[j][tt] = t
            ones_t = on_pool.tile([P, 8], BF16, tag="on")
            nc.sync.dma_start(ones_t[:], ones[:])

            # ---- stage A: kT[e, s] (bf16, e on partitions) ----
            kT_t = []
            for E in range(NE):
                t = kT_pool.tile([P, S], BF16, tag="kT")
                kT_t.append(t)
            for E in range(NE):
                for Sc in range(S // QB):
                    ps = ps_pool.tile([P, QB], F32, tag="ps")
                    for d in range(ND):
                        nc.tensor.matmul(
                            ps[:],
                            wk_t[d][:, E * P:(E + 1) * P],
                            xf[d][:, Sc * QB:(Sc + 1) * QB],
                            start=(d == 0), stop=(d == ND - 1),
                        )
                    nc.vector.tensor_copy(kT_t[E][:, Sc * QB:(Sc + 1) * QB], ps[:])

            # ---- stage B: v[s, e] (bf16, s on partitions) ----
            v_t = []
            for sT in range(NSK):
                t = v_pool.tile([P, D], BF16, tag="v")
                v_t.append(t)
            for sT in range(NSK):
                for ec in range(D // QB):
                    ps = ps_pool.tile([P, QB], F32, tag="ps")
                    for d in range(ND):
                        nc.tensor.matmul(
                            ps[:],
                            xf[d][:, sT * P:(sT + 1) * P],
                            wv_t[d][:, ec * QB:(ec + 1) * QB],
                            start=(d == 0), stop=(d == ND - 1),
                        )
                    nc.vector.tensor_copy(v_t[sT][:, ec * QB:(ec + 1) * QB], ps[:])

            # ---- stage C: qT[e, i] (q rows are cols 0:1024 of xf) ----
            qT_t = []
            for E in range(NE):
                t = qT_pool.tile([P, SQ], BF16, tag="qT")
                qT_t.append(t)
            for E in range(NE):
                for qc in range(SQ // QB):
                    ps = ps_pool.tile([P, QB], F32, tag="ps")
                    for d in range(ND):
                        nc.tensor.matmul(
                            ps[:],
                            wq_t[d][:, E * P:(E + 1) * P],
                            xf[d][:, qc * QB:(qc + 1) * QB],
                            start=(d == 0), stop=(d == ND - 1),
                        )
                    nc.vector.tensor_copy(qT_t[E][:, qc * QB:(qc + 1) * QB], ps[:])

            # ---- stage D: attention per q block ----
            for j in range(NQB):
                sk_list = _sk_list(j)
                cross = _cross_list(j)
                wtiles = {}
                for t in sk_list:
                    ps = ps_pool.tile([P, QB], F32, tag="ps")
                    for E in range(NE):
                        nc.tensor.matmul(
                            ps[:],
                            kT_t[E][:, t * P:(t + 1) * P],
                            qT_t[E][:, j * QB:(j + 1) * QB],
                            start=(E == 0), stop=(E == NE - 1),
                        )
                    wt = we_pool.tile([P, QB], BF16, tag="we")
                    nc.scalar.activation(wt[:], ps[:], AF.Exp, scale=float(SCALE))
                    if t in cross:
                        tt = cross.index(t)
                        nc.vector.tensor_mul(wt[:], wt[:], m_t[j][tt][:])
                    wtiles[t] = wt
                for u in range(QB // P):
                    av = av_pool.tile([P, D], F32, tag="av")
                    rs = rs_pool.tile([P, 1], F32, tag="rs")
                    n = len(sk_list)
                    for idx, t in enumerate(sk_list):
                        lhsT = wtiles[t][:, u * P:(u + 1) * P]
                        st, sp = idx == 0, idx == n - 1
                        nc.tensor.matmul(av[:, 0:QB], lhsT, v_t[t][:, 0:QB],
                                         start=st, stop=sp)
                        nc.tensor.matmul(av[:, QB:D], lhsT, v_t[t][:, QB:D],
                                         start=st, stop=sp)
                        nc.tensor.matmul(rs[:], lhsT, ones_t[:, 0:1],
                                         start=st, stop=sp)
                    rcp = sm_pool.tile([P, 1], F32, tag="rcp")
                    nc.vector.reciprocal(rcp[:], rs[:])
                    ot = o_pool.tile([P, D], F32, tag="o")
                    nc.vector.tensor_scalar_mul(ot[:], av[:], rcp[:])
                    r0 = (j * (QB // P) + u) * P
                    nc.sync.dma_start(out[r0:r0 + P, :], ot[:])

    nc.compile()
    return nc


def _prep_inputs(x, Wq, Wk, Wv):
    bf = ml_dtypes.bfloat16
    wq_b = np.ascontiguousarray(Wq.astype(bf))
    wk_b = np.ascontiguousarray(Wk.astype(bf))
    wv_b = np.ascontiguousarray(Wv.astype(bf))
    ones = np.ones((P, 8), bf)
    ks = np.arange(S)
    ii = np.arange(SQ)
    in_maps = []
    for c in range(NCORES):
        b, p = c // 2, c % 2
        xb = x[b]
        xp = np.concatenate([xb[p::2], xb[1 - p::2]], axis=0)  # [2048, 1024]
        xfT = np.ascontiguousarray(xp.T.astype(bf))
        # global key index of permuted key position
        gk = np.where(ks < SQ, 2 * ks + p, 2 * (ks - SQ) + (1 - p))
        gq = 2 * ii + p
        maskd = np.zeros((NQB, 8, P, QB), np.float32)
        for j in range(NQB):
            for tt, t in enumerate(_cross_list(j)):
                gk_t = gk[t * P:(t + 1) * P]
                gq_j = gq[QB * j:QB * (j + 1)]
                maskd[j, tt] = (gk_t[:, None] <= gq_j[None, :]).astype(np.float32)
        in_maps.append({
            "xfT": xfT, "wq": wq_b, "wk": wk_b, "wv": wv_b,
            "maskd": maskd.astype(bf), "ones": ones,
        })
    return in_maps


def kernel(x, Wq, Wk, Wv):
    global LAST_RESULT
    x = np.asarray(x, np.float32)
    Wq = np.asarray(Wq, np.float32)
    Wk = np.asarray(Wk, np.float32)
    Wv = np.asarray(Wv, np.float32)

    if "nc" not in _cache:
        _cache["nc"] = _build()
    nc = _cache["nc"]

    in_maps = _prep_inputs(x, Wq, Wk, Wv)
    res = run_bass_kernel_spmd(nc, in_maps, list(range(NCORES)), trace=TRACE)
    LAST_RESULT = res

    out = np.empty((B, S, D), np.float32)
    for c in range(NCORES):
        b, p = c // 2, c % 2
        out[b, p::2, :] = res.results[c]["out"]
    return out


# revision 4
# speedup vs baseline: 1.0690x; 1.0690x over previous
"""Causal single-head attention (B=4, S=2048, D=1024) on 8 NeuronCores.

Sharding: core c owns the q rows {2i + (c%2)} of batch c//2 (1024 rows).
Interleaving q rows by parity gives every core an identical causal
block structure, so one SPMD program serves all 8 cores; only the data
(and the staircase mask) differs per core.

Key order is globally redefined as [parity-0 rows asc, parity-1 rows
asc] — attention is invariant to key permutation as long as K, V and
the mask agree. Under that order each core's q rows are its own parity
half, its causal extent per q-block j is the uniform tile set
[0, 4(j+1)) + [8, 8+4(j+1)) (128-key tiles), and exactly 8 tiles per
block cross the diagonal (masked via a host-built staircase).

K/V projections are deduplicated across the core pair of each batch:
core p computes K/V only for its parity rows, then the pair exchanges
halves with a 2-core AllGather (DRAM bounce). Rank order of the gather
equals parity, so placement is uniform.

Device program per core (matmuls bf16, f32 PSUM accumulation):
  kT[e,s0]  = (xo @ Wk).T  --AllGather--> kT[e, 0:2048]
  v[s0,e]   = xo @ Wv      --AllGather--> v[0:2048, e]
  qT[e,i]   = (xo @ Wq).T
  scoresT[sk,sq] = kT-tile.T @ qT-tile   (8 acc matmuls per tile)
  w = exp(scoresT/32) * mask   (no max-subtraction: |scores/32| <~ 6)
  out[sq,e] = (w-tile.T @ v) / (w-tile.T @ ones)  (rowsum via N=1 matmul)
"""

import sys
import types

import numpy as np
import ml_dtypes

import concourse.tile as tile
from concourse import bacc, mybir
from concourse.bass_utils import run_bass_kernel_spmd


def _ensure_ntff_hook():
    """bass_utils imports antenv.axon_hooks when tracing; some containers
    lack that module. Register a process-local equivalent so trace=True
    works (or degrades to untraced instead of crashing)."""
    try:
        import antenv.axon_hooks  # noqa: F401
        return
    except ImportError:
        pass
    hook = None
    try:
        from trn_agent_boot.trn_boot import _ntff_profile_via_ctypes
        hook = _ntff_profile_via_ctypes("/opt/axon/libaxon_pjrt.so")
    except Exception:
        hook = None
    mod = types.ModuleType("antenv.axon_hooks")
    mod.get_axon_ntff_profile_hook = lambda: hook
    mod.set_axon_ntff_profile_hook = lambda h: None
    sys.modules["antenv.axon_hooks"] = mod


_ensure_ntff_hook()

BF16 = mybir.dt.bfloat16
F32 = mybir.dt.float32
AF = mybir.ActivationFunctionType

B, S, D = 4, 2048, 1024
P = 128
NCORES = 8
SQ = 1024            # q rows per core (= own parity half)
ND = D // P          # 8 contraction tiles over d
NE = D // P          # 8 tiles over e (d_out)
NSK = S // P         # 16 key tiles
QB = 512             # q-block width (matmul free dim)
NQB = SQ // QB       # 2 q blocks
SCALE = 1.0 / np.sqrt(np.float32(D))
PAIRS = [[2 * b, 2 * b + 1] for b in range(B)]

TRACE = False
LAST_RESULT = None

_cache = {}


def _sk_list(j):
    # key tiles needed by q-block j: prefix of each parity half
    return list(range(0, 4 * (j + 1))) + list(range(8, 8 + 4 * (j + 1)))


def _cross_list(j):
    # diagonal-crossing key tiles of q-block j (order matches maskd[j])
    return list(range(4 * j, 4 * (j + 1))) + list(range(8 + 4 * j, 8 + 4 * (j + 1)))


def _build():
    nc = bacc.Bacc("TRN2", target_bir_lowering=False, debug=False,
                   num_devices=NCORES)
    xoT = nc.dram_tensor("xoT", [D, SQ], BF16, kind="ExternalInput")
    wq = nc.dram_tensor("wq", [D, D], BF16, kind="ExternalInput")
    wk = nc.dram_tensor("wk", [D, D], BF16, kind="ExternalInput")
    wv = nc.dram_tensor("wv", [D, D], BF16, kind="ExternalInput")
    maskd = nc.dram_tensor("maskd", [NQB, 8, P, QB], BF16, kind="ExternalInput")
    ones = nc.dram_tensor("ones", [P, 8], BF16, kind="ExternalInput")
    out = nc.dram_tensor("out", [SQ, D], F32, kind="ExternalOutput")

    with tile.TileContext(nc) as tc:
        with (
            tc.tile_pool(name="xo", bufs=ND) as xo_pool,
            tc.tile_pool(name="w", bufs=2 * ND) as w_pool,
            tc.tile_pool(name="st", bufs=6) as st_pool,
            tc.tile_pool(name="kT", bufs=NE) as kT_pool,
            tc.tile_pool(name="v", bufs=NSK) as v_pool,
            tc.tile_pool(name="qT", bufs=NE) as qT_pool,
            tc.tile_pool(name="mk", bufs=2 * 8) as m_pool,
            tc.tile_pool(name="we", bufs=16) as we_pool,
            tc.tile_pool(name="on", bufs=1) as on_pool,
            tc.tile_pool(name="sm", bufs=4) as sm_pool,
            tc.tile_pool(name="o", bufs=2) as o_pool,
            tc.tile_pool(name="dr", bufs=4, space="DRAM") as dr_pool,
            tc.tile_pool(name="ps", bufs=2, space="PSUM") as ps_pool,
            tc.tile_pool(name="av", bufs=2, space="PSUM") as av_pool,
            tc.tile_pool(name="rs", bufs=2, space="PSUM") as rs_pool,
        ):
            # ---- input DMAs (xo + wk first: stage A's critical path) ----
            xo = []
            for d in range(ND):
                t = xo_pool.tile([P, SQ], BF16, tag="xo")
                nc.sync.dma_start(t[:], xoT[d * P:(d + 1) * P, :])
                xo.append(t)
            wk_t = []
            for d in range(ND):
                t = w_pool.tile([P, D], BF16, tag="w")
                nc.sync.dma_start(t[:], wk[d * P:(d + 1) * P, :])
                wk_t.append(t)
            wv_t = []
            for d in range(ND):
                t = w_pool.tile([P, D], BF16, tag="w")
                nc.sync.dma_start(t[:], wv[d * P:(d + 1) * P, :])
                wv_t.append(t)
            wq_t = []
            for d in range(ND):
                t = w_pool.tile([P, D], BF16, tag="w")
                nc.sync.dma_start(t[:], wq[d * P:(d + 1) * P, :])
                wq_t.append(t)
            m_t = [[None] * 8 for _ in range(NQB)]
            for j in range(NQB):
                for tt in range(8):
                    t = m_pool.tile([P, QB], BF16, tag="mk")
                    nc.sync.dma_start(t[:], maskd[j, tt, :, :])
                    m_t[j][tt] = t
            ones_t = on_pool.tile([P, 8], BF16, tag="on")
            nc.sync.dma_start(ones_t[:], ones[:])

            ex_in_k = dr_pool.tile([NE, P, SQ], BF16, tag="exik")
            ex_out_k = dr_pool.tile([2, NE, P, SQ], BF16, tag="exok")
            ex_in_v = dr_pool.tile([NE, P, D], BF16, tag="exiv")
            ex_out_v = dr_pool.tile([2, NE, P, D], BF16, tag="exov")

            # ---- stage A: kT own half [e, s0] -> exchange ----
            for E in range(NE):
                kst = st_pool.tile([P, SQ], BF16, tag="st")
                for Sc in range(SQ // QB):
                    ps = ps_pool.tile([P, QB], F32, tag="ps")
                    for d in range(ND):
                        nc.tensor.matmul(
                            ps[:],
                            wk_t[d][:, E * P:(E + 1) * P],
                            xo[d][:, Sc * QB:(Sc + 1) * QB],
                            start=(d == 0), stop=(d == ND - 1),
                        )
                    nc.vector.tensor_copy(kst[:, Sc * QB:(Sc + 1) * QB], ps[:])
                nc.sync.dma_start(ex_in_k[E], kst[:])
            nc.gpsimd.collective_compute(
                "AllGather", mybir.AluOpType.bypass, replica_groups=PAIRS,
                ins=[ex_in_k.opt()], outs=[ex_out_k.opt()],
            )
            kT_t = []
            for E in range(NE):
                t = kT_pool.tile([P, S], BF16, tag="kT")
                nc.sync.dma_start(t[:, 0:SQ], ex_out_k[0, E])
                nc.sync.dma_start(t[:, SQ:S], ex_out_k[1, E])
                kT_t.append(t)

            # ---- stage B: v own half [s0, e] -> exchange ----
            for sT in range(NE):
                vst = st_pool.tile([P, D], BF16, tag="st")
                for ec in range(D // QB):
                    ps = ps_pool.tile([P, QB], F32, tag="ps")
                    for d in range(ND):
                        nc.tensor.matmul(
                            ps[:],
                            xo[d][:, sT * P:(sT + 1) * P],
                            wv_t[d][:, ec * QB:(ec + 1) * QB],
                            start=(d == 0), stop=(d == ND - 1),
                        )
                    nc.vector.tensor_copy(vst[:, ec * QB:(ec + 1) * QB], ps[:])
                nc.sync.dma_start(ex_in_v[sT], vst[:])
            nc.gpsimd.collective_compute(
                "AllGather", mybir.AluOpType.bypass, replica_groups=PAIRS,
                ins=[ex_in_v.opt()], outs=[ex_out_v.opt()],
            )
            v_t = []
            for sT in range(NSK):
                t = v_pool.tile([P, D], BF16, tag="v")
                nc.sync.dma_start(t[:], ex_out_v[sT // NE, sT % NE])
                v_t.append(t)

            # ---- stage C: qT[e, i] from own rows ----
            qT_t = []
            for E in range(NE):
                t = qT_pool.tile([P, SQ], BF16, tag="qT")
                qT_t.append(t)
            for E in range(NE):
                for qc in range(SQ // QB):
                    ps = ps_pool.tile([P, QB], F32, tag="ps")
                    for d in range(ND):
                        nc.tensor.matmul(
                            ps[:],
                            wq_t[d][:, E * P:(E + 1) * P],
                            xo[d][:, qc * QB:(qc + 1) * QB],
                            start=(d == 0), stop=(d == ND - 1),
                        )
                    nc.vector.tensor_copy(qT_t[E][:, qc * QB:(qc + 1) * QB], ps[:])

            # ---- stage D: attention per q block ----
            for j in range(NQB):
                sk_list = _sk_list(j)
                cross = _cross_list(j)
                wtiles = {}
                for t in sk_list:
                    ps = ps_pool.tile([P, QB], F32, tag="ps")
                    for E in range(NE):
                        nc.tensor.matmul(
                            ps[:],
                            kT_t[E][:, t * P:(t + 1) * P],
                            qT_t[E][:, j * QB:(j + 1) * QB],
                            start=(E == 0), stop=(E == NE - 1),
                        )
                    wt = we_pool.tile([P, QB], BF16, tag="we")
                    nc.scalar.activation(wt[:], ps[:], AF.Exp, scale=float(SCALE))
                    if t in cross:
                        tt = cross.index(t)
                        nc.vector.tensor_mul(wt[:], wt[:], m_t[j][tt][:])
                    wtiles[t] = wt
                for u in range(QB // P):
                    av = av_pool.tile([P, D], F32, tag="av")
                    rs = rs_pool.tile([P, 1], F32, tag="rs")
                    n = len(sk_list)
                    for idx, t in enumerate(sk_list):
                        lhsT = wtiles[t][:, u * P:(u + 1) * P]
                        st, sp = idx == 0, idx == n - 1
                        nc.tensor.matmul(av[:, 0:QB], lhsT, v_t[t][:, 0:QB],
                                         start=st, stop=sp)
                        nc.tensor.matmul(av[:, QB:D], lhsT, v_t[t][:, QB:D],
                                         start=st, stop=sp)
                        nc.tensor.matmul(rs[:], lhsT, ones_t[:, 0:1],
                                         start=st, stop=sp)
                    rcp = sm_pool.tile([P, 1], F32, tag="rcp")
                    nc.vector.reciprocal(rcp[:], rs[:])
                    ot = o_pool.tile([P, D], F32, tag="o")
                    nc.vector.tensor_scalar_mul(ot[:], av[:], rcp[:])
                    r0 = (j * (QB // P) + u) * P
                    nc.sync.dma_start(out[r0:r0 + P, :], ot[:])

    nc.compile()
    return nc


def _prep_inputs(x, Wq, Wk, Wv):
    bf = ml_dtypes.bfloat16
    wq_b = np.ascontiguousarray(Wq.astype(bf))
    wk_b = np.ascontiguousarray(Wk.astype(bf))
    wv_b = np.ascontiguousarray(Wv.astype(bf))
    ones = np.ones((P, 8), bf)
    ks = np.arange(S)
    ii = np.arange(SQ)
    # global index of permuted key position (parity-0 rows, then parity-1)
    gk = np.where(ks < SQ, 2 * ks, 2 * (ks - SQ) + 1)
    in_maps = []
    for c in range(NCORES):
        b, p = c // 2, c % 2
        xoT = np.ascontiguousarray(x[b, p::2].T.astype(bf))  # [D, SQ]
        gq = 2 * ii + p
        maskd = np.zeros((NQB, 8, P, QB), np.float32)
        for j in range(NQB):
            for tt, t in enumerate(_cross_list(j)):
                gk_t = gk[t * P:(t + 1) * P]
                gq_j = gq[QB * j:QB * (j + 1)]
                maskd[j, tt] = (gk_t[:, None] <= gq_j[None, :]).astype(np.float32)
        in_maps.append({
            "xoT": xoT, "wq": wq_b, "wk": wk_b, "wv": wv_b,
            "maskd": maskd.astype(bf), "ones": ones,
        })
    return in_maps


def kernel(x, Wq, Wk, Wv):
    global LAST_RESULT
    x = np.asarray(x, np.float32)
    Wq = np.asarray(Wq, np.float32)
    Wk = np.asarray(Wk, np.float32)
    Wv = np.asarray(Wv, np.float32)

    if "nc" not in _cache:
        _cache["nc"] = _build()
    nc = _cache["nc"]

    in_maps = _prep_inputs(x, Wq, Wk, Wv)
    res = run_bass_kernel_spmd(nc, in_maps, list(range(NCORES)), trace=TRACE)
    LAST_RESULT = res

    out = np.empty((B, S, D), np.float32)
    for c in range(NCORES):
        b, p = c // 2, c % 2
        out[b, p::2, :] = res.results[c]["out"]
    return out


# revision 10
# speedup vs baseline: 1.3170x; 1.2320x over previous
"""Causal single-head attention (B=4, S=2048, D=1024) on 8 NeuronCores.

Sharding: core c owns the q rows {2i + (c%2)} of batch c//2 (1024 rows).
Interleaving q rows by parity gives every core an identical causal
block structure, so one SPMD program serves all 8 cores; only the data
(and the staircase mask) differs per core.

Key order is globally redefined as [parity-0 rows asc, parity-1 rows
asc] — attention is invariant to key permutation as long as K, V and
the mask agree. Under that order each core's q rows are its own parity
half, its causal extent per q-block j is the uniform tile set
[0, 4(j+1)) + [8, 8+4(j+1)) (128-key tiles), and exactly 8 tiles per
block cross the diagonal. Crossing tile with in-block offset c is
fully masked on its first 128*c q columns, so scores/exp are computed
only on the right 512-128c columns and the AV matmuls for q-subtiles
u < c are skipped.

K/V projections are deduplicated across the core pair of each batch:
core p computes K/V only for its parity rows, and the pair exchanges
halves with 2-core AllGathers (DRAM bounce), chunked so the collectives
and read-backs pipeline under the q projection and score matmuls.

Device program per core (matmuls bf16, f32 PSUM accumulation):
  kT[e,s0]  = (xo @ Wk).T  --AllGather (2 chunks)--> kT[e, 0:2048]
  v[s0,e]   = xo @ Wv      --AllGather (2 chunks)--> v[0:2048, e]
  qT[e,i]   = (xo @ Wq).T
  scoresT[sk,sq] = kT-tile.T @ qT-tile   (8 acc matmuls per tile)
  w = exp(scoresT/32) * mask   (no max-subtraction: |scores/32| <~ 6)
  out[sq,e] = (w-tile.T @ v) / (w-tile.T @ ones)  (rowsum via N=1 matmul)
"""

import sys
import types

import numpy as np
import ml_dtypes

import concourse.tile as tile
from concourse import bacc, mybir
from concourse.bass_utils import run_bass_kernel_spmd


def _ensure_ntff_hook():
    """bass_utils imports antenv.axon_hooks when tracing; some containers
    lack that module. Register a process-local equivalent so trace=True
    works (or degrades to untraced instead of crashing)."""
    try:
        import antenv.axon_hooks  # noqa: F401
        return
    except ImportError:
        pass
    hook = None
    try:
        from trn_agent_boot.trn_boot import _ntff_profile_via_ctypes
        hook = _ntff_profile_via_ctypes("/opt/axon/libaxon_pjrt.so")
    except Exception:
        hook = None
    mod = types.ModuleType("antenv.axon_hooks")
    mod.get_axon_ntff_profile_hook = lambda: hook
    mod.set_axon_ntff_profile_hook = lambda h: None
    sys.modules["antenv.axon_hooks"] = mod


_ensure_ntff_hook()

BF16 = mybir.dt.bfloat16
F32 = mybir.dt.float32
AF = mybir.ActivationFunctionType

B, S, D = 4, 2048, 1024
P = 128
NCORES = 8
SQ = 1024            # q rows per core (= own parity half)
ND = D // P          # 8 contraction tiles over d
NE = D // P          # 8 tiles over e (d_out)
NSK = S // P         # 16 key tiles
QB = 512             # q-block width (matmul free dim)
NQB = SQ // QB       # 2 q blocks
SCALE = 1.0 / np.sqrt(np.float32(D))
PAIRS = [[2 * b, 2 * b + 1] for b in range(B)]

TRACE = False
LAST_RESULT = None

_cache = {}


def _sk_list(j):
    # key tiles needed by q-block j: prefix of each parity half
    return list(range(0, 4 * (j + 1))) + list(range(8, 8 + 4 * (j + 1)))


def _cross_list(j):
    # diagonal-crossing key tiles of q-block j (order matches maskd[j])
    return list(range(4 * j, 4 * (j + 1))) + list(range(8 + 4 * j, 8 + 4 * (j + 1)))


def _coff(j, t):
    # in-block crossing offset: first 128*c q columns of tile t are fully
    # masked within q-block j (c = 0 for non-crossing computed tiles)
    return max(0, (t % 8) - 4 * j)


def _build():
    nc = bacc.Bacc("TRN2", target_bir_lowering=False, debug=False,
                   num_devices=NCORES)
    xoT = nc.dram_tensor("xoT", [D, SQ], BF16, kind="ExternalInput")
    wq = nc.dram_tensor("wq", [D, D], BF16, kind="ExternalInput")
    wk = nc.dram_tensor("wk", [D, D], BF16, kind="ExternalInput")
    wv = nc.dram_tensor("wv", [D, D], BF16, kind="ExternalInput")
    maskd = nc.dram_tensor("maskd", [NQB, 8, P, QB], BF16, kind="ExternalInput")
    ones = nc.dram_tensor("ones", [P, 8], BF16, kind="ExternalInput")
    out = nc.dram_tensor("out", [SQ, D], F32, kind="ExternalOutput")

    with tile.TileContext(nc) as tc:
        with (
            tc.tile_pool(name="xo", bufs=ND) as xo_pool,
            tc.tile_pool(name="wkp", bufs=ND) as wk_pool,
            tc.tile_pool(name="wvq", bufs=2) as wvq_pool,
            tc.tile_pool(name="st", bufs=3) as st_pool,
            tc.tile_pool(name="kT", bufs=1) as kT_pool,
            tc.tile_pool(name="v", bufs=1) as v_pool,
            tc.tile_pool(name="qT", bufs=NE) as qT_pool,
            tc.tile_pool(name="mk", bufs=1) as m_pool,
            tc.tile_pool(name="we", bufs=16) as we_pool,
            tc.tile_pool(name="on", bufs=1) as on_pool,
            tc.tile_pool(name="sm", bufs=4) as sm_pool,
            tc.tile_pool(name="o", bufs=2) as o_pool,
            tc.tile_pool(name="dr", bufs=8, space="DRAM") as dr_pool,
            tc.tile_pool(name="ps", bufs=2, space="PSUM") as ps_pool,
            tc.tile_pool(name="av", bufs=2, space="PSUM") as av_pool,
            tc.tile_pool(name="rs", bufs=2, space="PSUM") as rs_pool,
        ):
            # ---- input DMAs (xo + wk first: stage A's critical path) ----
            xo = []
            for d in range(ND):
                t = xo_pool.tile([P, SQ], BF16, tag="xo")
                nc.sync.dma_start(t[:], xoT[d * P:(d + 1) * P, :])
                xo.append(t)
            wk_t = []
            for d in range(ND):
                t = wk_pool.tile([P, D], BF16, tag="wk")
                nc.sync.dma_start(t[:], wk[d * P:(d + 1) * P, :])
                wk_t.append(t)
            # consolidated loads for everything off the critical path
            wv_big = wvq_pool.tile([P, ND, D], BF16, tag="wbig")
            nc.sync.dma_start(wv_big[:], wv.rearrange("(n p) m -> p n m", p=P))
            wq_big = wvq_pool.tile([P, ND, D], BF16, tag="wbig")
            nc.sync.dma_start(wq_big[:], wq.rearrange("(n p) m -> p n m", p=P))
            mask_big = m_pool.tile([P, NQB, 8, QB], BF16, tag="mk")
            nc.sync.dma_start(mask_big[:], maskd.rearrange("j t p m -> p j t m"))
            ones_t = on_pool.tile([P, 8], BF16, tag="on")
            nc.sync.dma_start(ones_t[:], ones[:])

            kT_big = kT_pool.tile([P, NE, S], BF16, tag="kT")
            v_big = v_pool.tile([P, NSK, D], BF16, tag="v")

            # ---- stage A: kT own half [e, s0], exchanged in 2 E-chunks ----
            for h in range(2):
                ex_in = dr_pool.tile([4, P, SQ], BF16, tag=f"exik{h}")
                ex_out = dr_pool.tile([2, 4, P, SQ], BF16, tag=f"exok{h}")
                for Eo in range(4):
                    E = 4 * h + Eo
                    kst = st_pool.tile([P, SQ], BF16, tag="st")
                    for Sc in range(SQ // QB):
                        ps = ps_pool.tile([P, QB], F32, tag="ps")
                        for d in range(ND):
                            nc.tensor.matmul(
                                ps[:],
                                wk_t[d][:, E * P:(E + 1) * P],
                                xo[d][:, Sc * QB:(Sc + 1) * QB],
                                start=(d == 0), stop=(d == ND - 1),
                            )
                        nc.vector.tensor_copy(
                            kst[:, Sc * QB:(Sc + 1) * QB], ps[:])
                    nc.sync.dma_start(ex_in[Eo], kst[:])
                nc.gpsimd.collective_compute(
                    "AllGather", mybir.AluOpType.bypass, replica_groups=PAIRS,
                    ins=[ex_in.opt()], outs=[ex_out.opt()],
                )
                for r in range(2):
                    nc.sync.dma_start(
                        kT_big[:, 4 * h:4 * (h + 1), r * SQ:(r + 1) * SQ],
                        ex_out[r].rearrange("n p m -> p n m"))

            # ---- stage B: v own half [s0, e], exchanged in 2 s-chunks ----
            for h in range(2):
                ex_in = dr_pool.tile([4, P, D], BF16, tag=f"exiv{h}")
                ex_out = dr_pool.tile([2, 4, P, D], BF16, tag=f"exov{h}")
                for so in range(4):
                    sT = 4 * h + so
                    vst = st_pool.tile([P, D], BF16, tag="st")
                    for ec in range(D // QB):
                        ps = ps_pool.tile([P, QB], F32, tag="ps")
                        for d in range(ND):
                            nc.tensor.matmul(
                                ps[:],
                                xo[d][:, sT * P:(sT + 1) * P],
                                wv_big[:, d, ec * QB:(ec + 1) * QB],
                                start=(d == 0), stop=(d == ND - 1),
                            )
                        nc.vector.tensor_copy(
                            vst[:, ec * QB:(ec + 1) * QB], ps[:])
                    nc.sync.dma_start(ex_in[so], vst[:])
                nc.gpsimd.collective_compute(
                    "AllGather", mybir.AluOpType.bypass, replica_groups=PAIRS,
                    ins=[ex_in.opt()], outs=[ex_out.opt()],
                )
                for r in range(2):
                    nc.sync.dma_start(
                        v_big[:, 8 * r + 4 * h:8 * r + 4 * (h + 1), :],
                        ex_out[r].rearrange("n p m -> p n m"))

            # ---- stage C: qT[e, i] from own rows ----
            qT_t = []
            for E in range(NE):
                t = qT_pool.tile([P, SQ], BF16, tag="qT")
                qT_t.append(t)
            for E in range(NE):
                for qc in range(SQ // QB):
                    ps = ps_pool.tile([P, QB], F32, tag="ps")
                    for d in range(ND):
                        nc.tensor.matmul(
                            ps[:],
                            wq_big[:, d, E * P:(E + 1) * P],
                            xo[d][:, qc * QB:(qc + 1) * QB],
                            start=(d == 0), stop=(d == ND - 1),
                        )
                    nc.vector.tensor_copy(qT_t[E][:, qc * QB:(qc + 1) * QB], ps[:])

            # ---- stage D: attention per q block ----
            for j in range(NQB):
                sk_list = _sk_list(j)
                cross = _cross_list(j)
                wtiles = {}
                for t in sk_list:
                    c = _coff(j, t)
                    w0 = c * P          # first live q column of this tile
                    ps = ps_pool.tile([P, QB], F32, tag="ps")
                    for E in range(NE):
                        nc.tensor.matmul(
                            ps[:, 0:QB - w0],
                            kT_big[:, E, t * P:(t + 1) * P],
                            qT_t[E][:, j * QB + w0:(j + 1) * QB],
                            start=(E == 0), stop=(E == NE - 1),
                        )
                    wt = we_pool.tile([P, QB], BF16, tag="we")
                    nc.scalar.activation(wt[:, w0:QB], ps[:, 0:QB - w0],
                                         AF.Exp, scale=float(SCALE))
                    if t in cross:
                        tt = cross.index(t)
                        nc.vector.tensor_mul(wt[:, w0:QB], wt[:, w0:QB],
                                             mask_big[:, j, tt, w0:QB])
                    wtiles[t] = wt
                for u in range(QB // P):
                    ts_u = [t for t in sk_list if _coff(j, t) <= u]
                    av = av_pool.tile([P, D], F32, tag="av")
                    rs = rs_pool.tile([P, 1], F32, tag="rs")
                    n = len(ts_u)
                    for idx, t in enumerate(ts_u):
                        lhsT = wtiles[t][:, u * P:(u + 1) * P]
                        st, sp = idx == 0, idx == n - 1
                        nc.tensor.matmul(av[:, 0:QB], lhsT, v_big[:, t, 0:QB],
                                         start=st, stop=sp)
                        nc.tensor.matmul(av[:, QB:D], lhsT, v_big[:, t, QB:D],
                                         start=st, stop=sp)
                        nc.tensor.matmul(rs[:], lhsT, ones_t[:, 0:1],
                                         start=st, stop=sp)
                    rcp = sm_pool.tile([P, 1], F32, tag="rcp")
                    nc.vector.reciprocal(rcp[:], rs[:])
                    ot = o_pool.tile([P, D], F32, tag="o")
                    nc.vector.tensor_scalar_mul(ot[:], av[:], rcp[:])
                    r0 = (j * (QB // P) + u) * P
                    nc.sync.dma_start(out[r0:r0 + P, :], ot[:])

    nc.compile()
    return nc


def _prep_inputs(x, Wq, Wk, Wv):
    bf = ml_dtypes.bfloat16
    wq_b = np.ascontiguousarray(Wq.astype(bf))
    wk_b = np.ascontiguousarray(Wk.astype(bf))
    wv_b = np.ascontiguousarray(Wv.astype(bf))
    ones = np.ones((P, 8), bf)
    ks = np.arange(S)
    ii = np.arange(SQ)
    # global index of permuted key position (parity-0 rows, then parity-1)
    gk = np.where(ks < SQ, 2 * ks, 2 * (ks - SQ) + 1)
    in_maps = []
    for c in range(NCORES):
        b, p = c // 2, c % 2
        xoT = np.ascontiguousarray(x[b, p::2].T.astype(bf))  # [D, SQ]
        gq = 2 * ii + p
        maskd = np.zeros((NQB, 8, P, QB), np.float32)
        for j in range(NQB):
            for tt, t in enumerate(_cross_list(j)):
                gk_t = gk[t * P:(t + 1) * P]
                gq_j = gq[QB * j:QB * (j + 1)]
                maskd[j, tt] = (gk_t[:, None] <= gq_j[None, :]).astype(np.float32)
        in_maps.append({
            "xoT": xoT, "wq": wq_b, "wk": wk_b, "wv": wv_b,
            "maskd": maskd.astype(bf), "ones": ones,
        })
    return in_maps


def kernel(x, Wq, Wk, Wv):
    global LAST_RESULT
    x = np.asarray(x, np.float32)
    Wq = np.asarray(Wq, np.float32)
    Wk = np.asarray(Wk, np.float32)
    Wv = np.asarray(Wv, np.float32)

    if "nc" not in _cache:
        _cache["nc"] = _build()
    nc = _cache["nc"]

    in_maps = _prep_inputs(x, Wq, Wk, Wv)
    res = run_bass_kernel_spmd(nc, in_maps, list(range(NCORES)), trace=TRACE)
    LAST_RESULT = res

    out = np.empty((B, S, D), np.float32)
    for c in range(NCORES):
        b, p = c // 2, c % 2
        out[b, p::2, :] = res.results[c]["out"]
    return out
